# revision 32
# baseline (speedup 1.0000x reference)
"""Trainium2 Bass kernel for nn_Corm (causal attention + corm any-reduction).

Strategy (8 NeuronCores, SPMD):
  - 64 (batch, head) pairs sharded 8 per core.
  - Per head: S^T = K @ Q^T computed per k-strip (k on partitions, q on free dim)
    in fp16 (scores fp32 in PSUM), exp on ScalarE -> pT (fp16, unnormalized,
    no max subtraction: |scores| <= ~8 so exp is safe in fp32/fp16 range).
  - PV: out[q, d] accumulated with lhsT = pT block (stationary), moving
    operand = [v | ones] so column 128 of the accumulator is the softmax
    denominator (free k-reduction).
  - Normalization: out * (1/denom) per q-row (tensor_scalar, per-partition).
  - corm[k] = any_q(p_un[k, q] >= t * denom[q]): threshold replicated across
    partitions via tiny PE matmuls, TT-subtract (2x fp16) + tensor_scalar
    is_ge with accum_out (4x) per k-strip.
  - The top-left corner (q < 128, k < 128) is recomputed in fp32 (QK fp32
    matmul + fp32 exp + exact fp32 denominator) because the corm margins
    there require better-than-fp16 precision; outside the corner margins are
    huge (verified against the reference margin distribution).
"""

import numpy as np

B, SEQ, H, D = 2, 2048, 32, 128
N_CORES = 8
HPC = (B * H) // N_CORES  # heads per core
NBLK = SEQ // 128  # 16
SCALE = 1.0 / float(np.sqrt(D))
VSTRIDE = 130  # v-block stride in vaug tile: 128 v cols + 1 ones col + 1 pad
OSTRIDE = 129  # out block stride: 128 out cols + denom col

_cache = {}


def _patch_tile_drain():
    """The pinned walrus rejects >2 sync waits on one CTRL (Drain)
    instruction; split the Tile kernel-tail drain into one single-wait
    drain per outstanding semaphore."""
    import concourse.tile as tile
    from concourse.vector_clock import ScopedClock, VectorClock

    def _drain_and_barrier_split(self, tick_clock, wait_clock):
        gc = tick_clock.global_clock
        n = len(gc)
        for i in range(n):
            v = gc[i]
            if v:
                vec = [0] * n
                vec[i] = v
                d = self.nc.sync.drain()
                wait_clock.add_sem_waits(d.ins, ScopedClock({None: VectorClock(vec)}))
        self.nc.all_engine_barrier()
        assert self.sems is not None
        popped = self.nc._tile_sem_poison_stack.pop()
        assert popped is self._sem_poison
        self.nc.clear_and_free_semaphores(list(self.sems.allocated().values()))
        self.nc.all_engine_barrier()

    tile.TileContext._drain_and_barrier = _drain_and_barrier_split


def _build_program():
    from contextlib import ExitStack

    import concourse.bass as bass
    import concourse.tile as tile
    from concourse import mybir

    dt = mybir.dt
    AO = mybir.AluOpType
    AF = mybir.ActivationFunctionType

    _patch_tile_drain()

    nc = bass.Bass("TRN2", target_bir_lowering=False, debug=False)

    Q = nc.dram_tensor("q", [HPC, SEQ, D], dt.float32, kind="ExternalInput")
    K = nc.dram_tensor("k", [HPC, SEQ, D], dt.float32, kind="ExternalInput")
    V = nc.dram_tensor("v", [HPC, SEQ, D], dt.float32, kind="ExternalInput")
    TM = nc.dram_tensor("tmask", [1, 1], dt.float32, kind="ExternalInput")
    ID32 = nc.dram_tensor("ident32", [128, 128], dt.float32, kind="ExternalInput")
    ID16 = nc.dram_tensor("ident16", [128, 128], dt.float16, kind="ExternalInput")
    MU16 = nc.dram_tensor("masku16", [128, 128], dt.float16, kind="ExternalInput")
    MU32 = nc.dram_tensor("masku32", [128, 128], dt.float32, kind="ExternalInput")
    SEL16 = nc.dram_tensor("sel16", [16, NBLK * 128], dt.float16, kind="ExternalInput")
    ONESC = nc.dram_tensor("onescol", [128, 1], dt.float32, kind="ExternalInput")
    ONESR = nc.dram_tensor("onesrow", [1, 128], dt.float32, kind="ExternalInput")

    OUT = nc.dram_tensor("out", [HPC, SEQ, D], dt.float32, kind="ExternalOutput")
    CORM = nc.dram_tensor("corm", [HPC, 128, NBLK], dt.float32, kind="ExternalOutput")

    # strip offsets in the pT buffer: strip j holds q columns [j*128, SEQ)
    span = [SEQ - j * 128 for j in range(NBLK)]
    off = np.concatenate([[0], np.cumsum(span[:-1])]).astype(int).tolist()
    ptw = int(sum(span))  # 17408

    with tile.TileContext(nc) as tc:
        with ExitStack() as ctx:
            con = ctx.enter_context(tc.tile_pool(name="con", bufs=1))
            big1 = ctx.enter_context(tc.tile_pool(name="big1", bufs=2))
            big1g = ctx.enter_context(tc.tile_pool(name="big1g", bufs=1))
            big2 = ctx.enter_context(tc.tile_pool(name="big2", bufs=2))
            smc = ctx.enter_context(tc.tile_pool(name="smc", bufs=2))
            pqk = ctx.enter_context(tc.tile_pool(name="pqk", bufs=4, space="PSUM"))
            ppv = ctx.enter_context(tc.tile_pool(name="ppv", bufs=2, space="PSUM"))
            prep = ctx.enter_context(tc.tile_pool(name="prep", bufs=1, space="PSUM"))

            # ---- constants ----
            ident32 = con.tile([128, 128], dt.float32)
            nc.sync.dma_start(ident32[:], ID32.ap()[:])
            ident16 = con.tile([128, 128], dt.float16)
            nc.sync.dma_start(ident16[:], ID16.ap()[:])
            masku16 = con.tile([128, 128], dt.float16)
            nc.sync.dma_start(masku16[:], MU16.ap()[:])
            masku32 = con.tile([128, 128], dt.float32)
            nc.sync.dma_start(masku32[:], MU32.ap()[:])
            sel16 = con.tile([16, NBLK * 128], dt.float16)
            nc.sync.dma_start(sel16[:], SEL16.ap()[:])
            onescol = con.tile([128, 1], dt.float32)
            nc.sync.dma_start(onescol[:], ONESC.ap()[:])
            onesrow = con.tile([1, 128], dt.float32)
            nc.sync.dma_start(onesrow[:], ONESR.ap()[:])
            tm_sb = con.tile([1, 1], dt.float32)
            nc.sync.dma_start(tm_sb[:], TM.ap()[:])

            # t replicated to all 128 partitions: ones[1,128].T @ t[1,1]
            t_ps = ppv.tile([128, 512], dt.float32, tag="pv")
            nc.tensor.matmul(t_ps[:, 0:1], onesrow[:], tm_sb[:], start=True, stop=True)
            t_sb = con.tile([128, 1], dt.float32)
            nc.vector.tensor_copy(t_sb[:], t_ps[:, 0:1])

            # corners (k<128, q<128) exact in fp32; computed inside each
            # head's prologue so the work spreads across the timeline
            cntc_all = con.tile([128, HPC], dt.float32)
            cnt_all = con.tile([128, HPC * NBLK], dt.float32)

            def corner(h):
                qc = smc.tile([128, 128], dt.float32, tag="qc")
                nc.sync.dma_start(qc[:], Q.ap()[h, 0:128, :])
                kc = smc.tile([128, 128], dt.float32, tag="kc")
                nc.sync.dma_start(kc[:], K.ap()[h, 0:128, :])
                ct1 = ppv.tile([128, 512], dt.float32, tag="pv")
                nc.tensor.transpose(ct1[:, 0:128], qc[:], ident32[:])
                qTc = smc.tile([128, 128], dt.float32, tag="qTc")
                nc.vector.tensor_copy(qTc[:], ct1[:, 0:128])
                ct2 = ppv.tile([128, 512], dt.float32, tag="pv")
                nc.tensor.transpose(ct2[:, 0:128], kc[:], ident32[:])
                kTc = smc.tile([128, 128], dt.float32, tag="kTc")
                nc.vector.tensor_copy(kTc[:], ct2[:, 0:128])

                cps = ppv.tile([128, 512], dt.float32, tag="pv")
                nc.tensor.matmul(cps[:, 0:128], kTc[:], qTc[:], start=True, stop=False)
                nc.tensor.matmul(cps[:, 0:128], masku32[:], ident32[:], start=False, stop=True)
                pc32 = smc.tile([128, 128], dt.float32, tag="pc32")
                nc.scalar.activation(pc32[:], cps[:, 0:128], AF.Exp, scale=SCALE)

                dc_ps = ppv.tile([128, 512], dt.float32, tag="pv")
                nc.tensor.matmul(dc_ps[0:1, 0:128], onescol[:], pc32[:], start=True, stop=True)
                tcrow = smc.tile([1, 128], dt.float32, tag="tcrow")
                nc.vector.tensor_scalar(
                    tcrow[:], dc_ps[0:1, 0:128], tm_sb[:], None, op0=AO.mult
                )
                crep_ps = ppv.tile([128, 512], dt.float32, tag="pv")
                nc.tensor.matmul(crep_ps[:, 0:128], onesrow[:], tcrow[:], start=True, stop=True)
                zc = smc.tile([128, 128], dt.float32, tag="zc")
                nc.vector.tensor_tensor(zc[:], pc32[:], crep_ps[:, 0:128], op=AO.subtract)
                gc = smc.tile([128, 128], dt.float32, tag="gc")
                nc.vector.tensor_scalar(
                    gc[:], zc[:], 0.0, None, op0=AO.is_ge, op1=AO.add,
                    accum_out=cntc_all[:, h : h + 1],
                )

            def prologue(h):
                st = {}
                q32 = big1.tile([128, NBLK * 128], dt.float32, tag="q32")
                k32 = big1.tile([128, NBLK * 128], dt.float32, tag="k32")
                v32 = big1.tile([128, NBLK * 128], dt.float32, tag="v32")
                nc.sync.dma_start(
                    q32[:].rearrange("p (b d) -> p b d", d=128),
                    Q.ap()[h].rearrange("(b p) d -> p b d", p=128),
                )
                nc.sync.dma_start(
                    k32[:].rearrange("p (b d) -> p b d", d=128),
                    K.ap()[h].rearrange("(b p) d -> p b d", p=128),
                )
                nc.sync.dma_start(
                    v32[:].rearrange("p (b d) -> p b d", d=128),
                    V.ap()[h].rearrange("(b p) d -> p b d", p=128),
                )

                q16 = big2.tile([128, NBLK * 128], dt.float16, tag="qk16")
                nc.vector.tensor_copy(q16[:], q32[:])
                k16 = big2.tile([128, NBLK * 128], dt.float16, tag="qk16")
                nc.vector.tensor_copy(k16[:], k32[:])

                vaug = big2.tile([128, NBLK * VSTRIDE], dt.float16, tag="vaug")
                vview = vaug[:].rearrange("p (b c) -> p b c", c=VSTRIDE)
                nc.vector.tensor_copy(
                    vview[:, :, 0:128],
                    v32[:].rearrange("p (b c) -> p b c", c=128),
                )
                nc.gpsimd.memset(vview[:, :, 128:129], 1.0)

                qT = big2.tile([128, NBLK * 128], dt.float16, tag="qT")
                kT = big2.tile([128, NBLK * 128], dt.float16, tag="kT")
                # descending order: strip 15 (emitted first) needs only blk 15
                for blk in reversed(range(NBLK)):
                    s = blk * 128
                    nc.sync.dma_start_transpose(kT[:, s : s + 128], k16[:, s : s + 128])
                    nc.sync.dma_start_transpose(qT[:, s : s + 128], q16[:, s : s + 128])

                st["qT"], st["kT"], st["vaug"] = qT, kT, vaug
                pT = big2.tile([128, ptw], dt.float16, tag="pT")
                st["pT"] = pT
                outsb = big2.tile([128, NBLK * OSTRIDE], dt.float32, tag="outsb")
                st["outsb"] = outsb
                return st

            def head_main(h, st_):
                qT, kT, pT = st_["qT"], st_["kT"], st_["pT"]
                vaug, outsb = st_["vaug"], st_["outsb"]
                oview = outsb[:].rearrange("p (b c) -> p b c", c=OSTRIDE)
                dcols = oview[:, :, 128]  # [128, 16] denominators
                r32 = smc.tile([128, NBLK], dt.float32, tag="r32")

                def pv_group(g0, g1):
                    # PV bursts, one q-block per PSUM slot; ACT drains trickle
                    for i in range(g0, g1):
                        pv = ppv.tile([128, OSTRIDE], dt.float32, tag="pv")
                        for jj in range(i + 1):
                            nc.tensor.matmul(
                                pv[:],
                                pT[:, off[jj] + (i - jj) * 128 : off[jj] + (i - jj) * 128 + 128],
                                vaug[:, jj * VSTRIDE : jj * VSTRIDE + OSTRIDE],
                                start=(jj == 0),
                                stop=(jj == i),
                            )
                        nc.scalar.copy(
                            outsb[:, i * OSTRIDE : (i + 1) * OSTRIDE], pv[:]
                        )

                for j in range(NBLK):
                    sp = span[j]
                    for cs in range(0, sp, 512):
                        cl = min(512, sp - cs)
                        stt = pqk.tile([128, 512], dt.float32, tag="st")
                        for ns in range(0, cl, 512):
                            nl = min(512, cl - ns)
                            first = (cs == 0 and ns == 0)
                            nc.tensor.matmul(
                                stt[:, ns : ns + nl],
                                kT[:, j * 128 : j * 128 + 128],
                                qT[:, j * 128 + cs + ns : j * 128 + cs + ns + nl],
                                start=True,
                                stop=not first,
                            )
                        if cs == 0:
                            # causal mask: adds -30000 where k_local > q_local
                            nc.tensor.matmul(
                                stt[:, 0:128],
                                masku16[:],
                                ident16[:],
                                start=False,
                                stop=True,
                            )
                        nc.scalar.activation(
                            pT[:, off[j] + cs : off[j] + cs + cl],
                            stt[:, 0:cl],
                            AF.Exp,
                            scale=SCALE,
                        )
                    # PV one strip delayed: block j-1's last strip (j-1)
                    # was exp'd during this strip's QK, so the PE never
                    # stalls on the exp semaphore
                    if j > 0:
                        pv_group(j - 1, j)
                pv_group(NBLK - 1, NBLK)

                # batched normalize at head end
                nc.vector.reciprocal(r32[:], dcols)
                for i in range(NBLK):
                    nc.vector.tensor_scalar(
                        outsb[:, i * OSTRIDE : i * OSTRIDE + 128],
                        outsb[:, i * OSTRIDE : i * OSTRIDE + 128],
                        r32[:, i : i + 1],
                        None,
                        op0=AO.mult,
                    )
                    nc.gpsimd.dma_start(
                        OUT.ap()[h, i * 128 : (i + 1) * 128, :],
                        outsb[:, i * OSTRIDE : i * OSTRIDE + 128],
                    )

            def corm_post(h, st_):
                pT = st_["pT"]
                outsb = st_["outsb"]
                dcols = outsb[:].rearrange("p (b c) -> p b c", c=OSTRIDE)[:, :, 128]
                # ---- corm thresholds: rep[k, q] = t * denom[q] ----
                dT_ps = ppv.tile([128, 512], dt.float32, tag="pv")
                nc.tensor.transpose(dT_ps[0:16, 0:128], dcols, ident32[:])
                dT16 = smc.tile([16, 128], dt.float16, tag="dT16")
                nc.scalar.activation(
                    dT16[:], dT_ps[0:16, 0:128], AF.Copy, scale=t_sb[0:16, :]
                )
                rep = big1g.tile([128, NBLK * 128], dt.float16, tag="rep")
                for half in range(2):
                    rp = prep.tile([128, 1024], dt.float32, tag="rp")
                    for ii in range(8):
                        i = half * 8 + ii
                        nc.tensor.matmul(
                            rp[:, ii * 128 : (ii + 1) * 128],
                            sel16[:, i * 128 : (i + 1) * 128],
                            dT16[:],
                            start=True,
                            stop=True,
                        )
                    nc.vector.tensor_copy(rep[:, half * 1024 : (half + 1) * 1024], rp[:])

                # ---- corm main pass (excludes corner q<128 for strip 0) ----
                cnt = cnt_all[:, h * NBLK : (h + 1) * NBLK]
                z = big1g.tile([128, NBLK * 128], dt.float16, tag="z")
                g = big1g.tile([128, NBLK * 128], dt.float16, tag="g")
                for j in range(NBLK):
                    qs = 128 if j == 0 else 0
                    w = span[j] - qs
                    nc.vector.tensor_tensor(
                        z[:, 0:w],
                        pT[:, off[j] + qs : off[j] + span[j]],
                        rep[:, j * 128 + qs : SEQ],
                        op=AO.subtract,
                    )
                    nc.vector.tensor_scalar(
                        g[:, 0:w],
                        z[:, 0:w],
                        0.0,
                        None,
                        op0=AO.is_ge,
                        op1=AO.add,
                        accum_out=cnt[:, j : j + 1],
                    )


            # software-pipelined emission: prologue two heads ahead, emitted
            # after head h's corm batch so casts don't head-of-line block it
            sts = {0: prologue(0)}
            if HPC > 1:
                sts[1] = prologue(1)
            for h in range(HPC):
                head_main(h, sts[h])
                corm_post(h, sts[h])
                if h + 2 < HPC:
                    sts[h + 2] = prologue(h + 2)
                del sts[h]

            # corners at the tail: fill the pipeline wind-down, then merge
            # their counts and store corm
            for h in range(HPC):
                corner(h)
            for h in range(HPC):
                cnt = cnt_all[:, h * NBLK : (h + 1) * NBLK]
                nc.vector.tensor_tensor(
                    cnt[:, 0:1], cnt[:, 0:1], cntc_all[:, h : h + 1], op=AO.add
                )
                nc.gpsimd.dma_start(CORM.ap()[h], cnt[:])

    _split_patch(nc)
    return nc


def _split_patch(nc):
    import orjson

    orig = nc.to_json_bytes

    def patched():
        j = orjson.loads(orig())
        ctr = 0
        for f in j["functions"]:
            for blk in f["blocks"]:
                out = []
                for inst in blk["instructions"]:
                    si = inst.get("sync_info")
                    ow = si.get("on_wait") if si else None
                    if ow and len(ow) > 1:
                        for w in ow[:-1]:
                            ctr += 1
                            out.append(
                                {
                                    "debug": inst.get("debug", 0),
                                    "engine": inst["engine"],
                                    "ins": [],
                                    "outs": [],
                                    "is_reset_sema": False,
                                    "name": f"WS-{ctr}",
                                    "opcode": "Drain",
                                    "sync_info": {"on_update": [], "on_wait": [w]},
                                }
                            )
                        si["on_wait"] = [ow[-1]]
                    out.append(inst)
                blk["instructions"] = out
        return orjson.dumps(j)

    nc.to_json_bytes = patched


def _get_program():
    if "nc" not in _cache:
        _cache["nc"] = _build_program()
    return _cache["nc"]


def _consts():
    if "consts" in _cache:
        return _cache["consts"]
    ident32 = np.eye(128, dtype=np.float32)
    ident16 = np.eye(128, dtype=np.float16)
    r = np.arange(128)
    # masku[c, m] = -30000 if m > c else 0 ; diag-mask matmul adds
    # masku.T[i, j] = -30000 where i > j (k_local > q_local)
    masku16 = np.where(r[None, :] > r[:, None], np.float16(-30000.0), np.float16(0.0))
    masku32 = np.where(r[None, :] > r[:, None], np.float32(-1e9), np.float32(0.0))
    sel16 = np.zeros((16, NBLK * 128), dtype=np.float16)
    for i in range(NBLK):
        sel16[i, i * 128 : (i + 1) * 128] = 1.0
    onescol = np.ones((128, 1), dtype=np.float32)
    onesrow = np.ones((1, 128), dtype=np.float32)
    _cache["consts"] = dict(
        ident32=ident32,
        ident16=ident16,
        masku16=masku16,
        masku32=masku32,
        sel16=sel16,
        onescol=onescol,
        onesrow=onesrow,
    )
    return _cache["consts"]


def kernel(q, k, v, corm_mask, _return_raw=False):
    from concourse.bass_utils import run_bass_kernel_spmd

    q = np.asarray(q)
    k = np.asarray(k)
    v = np.asarray(v)
    t = np.asarray(corm_mask).reshape(1, 1).astype(np.float32)

    nc = _get_program()
    consts = _consts()

    # [B, S, H, D] -> [B*H, S, D]
    qf = np.ascontiguousarray(np.transpose(q, (0, 2, 1, 3))).reshape(B * H, SEQ, D)
    kf = np.ascontiguousarray(np.transpose(k, (0, 2, 1, 3))).reshape(B * H, SEQ, D)
    vf = np.ascontiguousarray(np.transpose(v, (0, 2, 1, 3))).reshape(B * H, SEQ, D)

    in_maps = []
    for c in range(N_CORES):
        sl = slice(c * HPC, (c + 1) * HPC)
        in_maps.append(
            dict(
                q=np.ascontiguousarray(qf[sl]),
                k=np.ascontiguousarray(kf[sl]),
                v=np.ascontiguousarray(vf[sl]),
                tmask=t,
                **consts,
            )
        )

    res = run_bass_kernel_spmd(nc, in_maps, list(range(N_CORES)))
    out_all = np.concatenate([res.results[c]["out"] for c in range(N_CORES)], axis=0)
    corm_all = np.concatenate([res.results[c]["corm"] for c in range(N_CORES)], axis=0)

    out = np.transpose(out_all.reshape(B, H, SEQ, D), (0, 2, 1, 3))
    # counts [B*H, 128, 16] -> bool [B, H, SEQ] with seq = blk*128 + p
    corm = (np.transpose(corm_all, (0, 2, 1)).reshape(B, H, SEQ) >= 0.5)
    if _return_raw:
        return (np.ascontiguousarray(out), corm, res)
    return (np.ascontiguousarray(out), corm)
